# revision 1
# baseline (speedup 1.0000x reference)
"""Multi-head self-attention (B=2, S=2048, D=1024, H=16, causal+padding mask)
on 8 Trainium2 NeuronCores via Bass/Tile, SPMD.

Sharding: core c -> batch b = c//4, query residue r = c%4. Each core computes
the full K/V projections for its batch (duplicated across the 4 cores of a
batch -- cheaper than any cross-core collective at this size) and attention +
output projection for the strided query set q = 4j + r, j = 0..511. Strided
(rather than blocked) query assignment keeps the causal loop structure
identical on every core, which SPMD requires; per-core differences (mask
values, packed activations) travel as data.

Dataflow is fully transposed so no on-chip transposes are needed:
  QT[dh, q]   = (Wq x^T)/8 + bq/8          lhsT = Wq^T chunks, rhs = xq^T
  KT[dh, k]   = Wk x^T + bk
  V [k, dh]   = x Wv^T  (+ ones column)    lhsT = x^T chunks,  rhs = Wv^T
  ST[k, q]    = KT_h^T QT_h  (per head)
  E           = exp(ST + padmask_bias) * causal01
  OT'[dh+1,q] = V_aug^T E   (row 64 = softmax denominators r[q])
  AT[dh, q]   = OT' * (1/r)                (broadcast r via rank-1 matmul,
                                            then reciprocal on all 64 lanes)
  YT[n, q]    = Wo_arr^T AT + (bo + Wo bv) lhsT = Wo^T chunks, rhs = AT
Matmuls run in float32r (full-rate fp32 with reduced mantissa, ~1e-4 rel
error). Softmax skips max-subtraction: scores are bounded (|S| < ~5) so exp
is safe, and masked lanes get -1e4 added pre-exp which underflows to exactly
0 after exp.
"""

import sys

if "/opt/trn_rl_repo" not in sys.path:
    sys.path.insert(0, "/opt/trn_rl_repo")

import numpy as np

B, S, D, H, HD = 2, 2048, 1024, 16, 64
N_CORES = 8
JQ = S // 4          # 512 packed queries per core
MC = D // 128        # 8 contraction chunks of 128
NKT = S // 128       # 16 key tiles
JB_N = 256           # packed query block (matmul N)

_CACHE = {}


def _split_waits(nc, mybir):
    """This walrus build accepts only one sync-wait per instruction; move
    extra waits onto NOPs inserted just before, on the same engine."""
    n_new = 0
    for f in nc.m.functions:
        for blk in f.blocks:
            out = []
            for inst in blk.instructions:
                si = inst.sync_info
                if si is not None and si.on_wait is not None and len(si.on_wait) > 1:
                    waits = list(si.on_wait)
                    for w in waits[:-1]:
                        n_new += 1
                        out.append(mybir.InstNoOp(
                            name=f"I-waitsplit-{n_new}",
                            engine=inst.engine,
                            ins=[], outs=[],
                            sync_info=mybir.SyncInfo(on_wait=[w], on_update=[]),
                        ))
                    inst.sync_info = mybir.SyncInfo(
                        on_wait=[waits[-1]], on_update=list(si.on_update or []))
                out.append(inst)
            blk.instructions[:] = out
    return n_new


def _build():
    import concourse.bass as bass
    import concourse.mybir as mybir
    import concourse.tile as tile
    from contextlib import ExitStack

    f32 = mybir.dt.float32
    f32r = mybir.dt.float32r
    EXP = mybir.ActivationFunctionType.Exp
    IDENT = mybir.ActivationFunctionType.Identity
    COPY = mybir.ActivationFunctionType.Copy

    nc = bass.Bass()
    xT = nc.declare_dram_parameter("xT", [D, S], f32r, isOutput=False)
    xqT = nc.declare_dram_parameter("xqT", [D, JQ], f32r, isOutput=False)
    wqT = nc.declare_dram_parameter("wqT", [D, D], f32r, isOutput=False)
    wkT = nc.declare_dram_parameter("wkT", [D, D], f32r, isOutput=False)
    wvT = nc.declare_dram_parameter("wvT", [D, D], f32r, isOutput=False)
    woT = nc.declare_dram_parameter("woT", [D, D], f32r, isOutput=False)
    bq8 = nc.declare_dram_parameter("bq8", [D], f32, isOutput=False)
    bkv = nc.declare_dram_parameter("bk", [D], f32, isOutput=False)
    obias = nc.declare_dram_parameter("obias", [D], f32, isOutput=False)
    pmb = nc.declare_dram_parameter("pmb", [S], f32, isOutput=False)
    cmask = nc.declare_dram_parameter("cmask", [8, 128, JB_N], f32r, isOutput=False)
    onesc = nc.declare_dram_parameter("onesc", [1, HD], f32r, isOutput=False)
    out = nc.declare_dram_parameter("o", [D, JQ], f32, isOutput=True)

    with tile.TileContext(nc) as tc, ExitStack() as ctx, \
            nc.allow_low_precision("fp32r matmul inputs keep ~19 bits"):
        ec = ctx.enter_context
        consts = ec(tc.tile_pool(name="consts", bufs=1))
        big = ec(tc.tile_pool(name="big", bufs=1))
        e_p = ec(tc.tile_pool(name="e", bufs=6))
        rc_p = ec(tc.tile_pool(name="rc", bufs=1))
        rb_p = ec(tc.tile_pool(name="rb", bufs=1))
        yt_p = ec(tc.tile_pool(name="yt", bufs=2))
        proj_ps = ec(tc.tile_pool(name="proj_ps", bufs=2, space="PSUM"))
        st_ps = ec(tc.tile_pool(name="st_ps", bufs=3, space="PSUM"))
        ot_ps = ec(tc.tile_pool(name="ot_ps", bufs=3, space="PSUM"))

        # ---- constants into SBUF ----
        bq8_sb = consts.tile([128, MC], f32, tag="bq8")
        nc.sync.dma_start(out=bq8_sb, in_=bq8.rearrange("(c p) -> p c", p=128))
        bk_sb = consts.tile([128, MC], f32, tag="bk")
        nc.sync.dma_start(out=bk_sb, in_=bkv.rearrange("(c p) -> p c", p=128))
        ob_sb = consts.tile([128, MC], f32, tag="ob")
        nc.sync.dma_start(out=ob_sb, in_=obias.rearrange("(c p) -> p c", p=128))
        pmb_sb = consts.tile([128, NKT], f32, tag="pmb")
        nc.sync.dma_start(out=pmb_sb, in_=pmb.rearrange("(t p) -> p t", p=128))
        cm_sb = consts.tile([128, 8, JB_N], f32r, tag="cm")
        nc.sync.dma_start(out=cm_sb, in_=cmask.rearrange("t p j -> p t j"))
        ones_sb = consts.tile([1, HD], f32r, tag="ones")
        nc.sync.dma_start(out=ones_sb, in_=onesc[:, :])

        # persistent activations
        QT_sb = big.tile([128, MC, JQ], f32r, tag="qt")            # 16KB/part
        KT_sb = big.tile([128, MC, S], f32r, tag="kt")             # 64KB/part
        V_sb = big.tile([128, NKT, H, HD + 1], f32r, tag="v")      # 66.6KB/part
        # xq (Q-proj phase) and AT (attention/output phases) have disjoint
        # lifetimes; share one 16KB slot via a common tag.
        xq_sb = big.tile([128, MC, JQ], f32r, tag="xqat")
        MULT = mybir.AluOpType.mult
        ADD = mybir.AluOpType.add
        xre = xT.rearrange("(c p) k -> p c k", p=128)
        wkre = wkT.rearrange("(c p) n -> p c n", p=128)
        wvre = wvT.rearrange("(c p) n -> p c n", p=128)
        AT_sb = None

        def attention_pair(h0, jb, kt_lo, kt_hi, otps):
            """Emit S^T/exp/mask/PV for heads (h0, h0+1), query block jb,
            key tiles [kt_lo, kt_hi), interleaved for PE overlap. otps holds
            the two accumulation psum tiles (allocated at kt_lo==0)."""
            nkt = 8 if jb == 0 else 16
            for kt in range(kt_lo, kt_hi):
                for hi in range(2):
                    h = h0 + hi
                    pr, hw = h // 2, 64 * (h % 2)
                    st = st_ps.tile([128, JB_N], f32, tag="st")
                    nc.tensor.matmul(
                        st[:],
                        KT_sb[hw:hw + 64, pr, kt * 128:(kt + 1) * 128],
                        QT_sb[hw:hw + 64, pr, jb * JB_N:(jb + 1) * JB_N],
                        start=True, stop=True)
                    e = e_p.tile([128, JB_N], f32r, tag="e")
                    nc.scalar.activation(out=e[:], in_=st[:], func=EXP,
                                         bias=pmb_sb[:, kt:kt + 1])
                    tp = kt - 8 * jb
                    if tp >= 0:
                        nc.gpsimd.tensor_mul(e[:], e[:], cm_sb[:, tp, :])
                    nc.tensor.matmul(otps[hi][:], V_sb[:, kt, h, :], e[:],
                                     start=(kt == 0), stop=(kt == nkt - 1))

        def attention_norm(h0, jb, otps):
            for hi in range(2):
                h = h0 + hi
                pr, hw = h // 2, 64 * (h % 2)
                otp = otps[hi]
                rc = rc_p.tile([1, JB_N], f32r, tag="rc")
                nc.scalar.activation(out=rc[:], in_=otp[HD:HD + 1, :], func=COPY)
                bc = st_ps.tile([HD, JB_N], f32, tag="st")
                nc.tensor.matmul(bc[:], ones_sb[:], rc[:], start=True, stop=True)
                rb = rb_p.tile([HD, JB_N], f32, tag="rb")
                nc.vector.reciprocal(out=rb[:], in_=bc[:])
                nc.vector.tensor_mul(
                    AT_sb[hw:hw + 64, pr, jb * JB_N:(jb + 1) * JB_N],
                    otp[0:HD, :], rb[:])

        def attention(h0, kt_done):
            """Full attention for heads (h0, h0+1) given KT/V ready up to
            kt_done; emits both query blocks."""
            for jb in range(2):
                nkt = 8 if jb == 0 else 16
                otp_a = ot_ps.tile([HD + 1, JB_N], f32, tag="ot")
                otp_b = ot_ps.tile([HD + 1, JB_N], f32, tag="ot")
                otps = [otp_a, otp_b]
                attention_pair(h0, jb, 0, min(nkt, kt_done), otps)
                if kt_done < nkt:
                    attention_pair(h0, jb, kt_done, nkt, otps)
                attention_norm(h0, jb, otps)

        # ---- Q projection: QT[dh, jq] = Wq x^T /8 + bq/8 ----
        nc.sync.dma_start(out=xq_sb,
                          in_=xqT.rearrange("(c p) j -> p c j", p=128))
        with tc.tile_pool(name="wq", bufs=2) as wq_p:
            for q4 in range(4):
                wq_sb = wq_p.tile([128, MC, 256], f32r, tag="wq")
                nc.sync.dma_start(
                    out=wq_sb,
                    in_=wqT.rearrange("(c p) n -> p c n", p=128)[:, :, q4 * 256:(q4 + 1) * 256])
                for dt_i in range(2):
                    dt_ = q4 * 2 + dt_i
                    ps = proj_ps.tile([128, 512], f32, tag="ps")
                    for m in range(MC):
                        nc.tensor.matmul(
                            ps[:], wq_sb[:, m, dt_i * 128:(dt_i + 1) * 128],
                            xq_sb[:, m, :],
                            start=(m == 0), stop=(m == MC - 1))
                    nc.vector.tensor_scalar(
                        out=QT_sb[:, dt_, :], in0=ps[:],
                        scalar1=0.125, scalar2=bq8_sb[:, dt_:dt_ + 1],
                        op0=MULT, op1=ADD)

        AT_sb = big.tile([128, MC, JQ], f32r, tag="xqat")

        # ---- K/V projections split by dh half (head groups 0-7 / 8-15) so
        # attention on the first half overlaps the second half's projections.
        for half in range(2):
            # K rows for pairs [4*half, 4*half+4)
            with tc.tile_pool(name="kproj", bufs=1) as kp, \
                    tc.tile_pool(name="kw", bufs=2) as kwp:
                for kb8 in range(8):
                    xt_sb = kp.tile([128, MC, 256], f32r, tag="xt")
                    nc.sync.dma_start(
                        out=xt_sb, in_=xre[:, :, kb8 * 256:(kb8 + 1) * 256])
                    for q4 in (2 * half, 2 * half + 1):
                        wk_sb = kwp.tile([128, MC, 256], f32r, tag="wk")
                        nc.sync.dma_start(
                            out=wk_sb, in_=wkre[:, :, q4 * 256:(q4 + 1) * 256])
                        for dt_i in range(2):
                            dt_ = q4 * 2 + dt_i
                            ps = proj_ps.tile([128, 256], f32, tag="ps")
                            for m in range(MC):
                                nc.tensor.matmul(
                                    ps[:], wk_sb[:, m, dt_i * 128:(dt_i + 1) * 128],
                                    xt_sb[:, m, :],
                                    start=(m == 0), stop=(m == MC - 1))
                            nc.vector.tensor_scalar_add(
                                out=KT_sb[:, dt_, kb8 * 256:(kb8 + 1) * 256],
                                in0=ps[:], scalar1=bk_sb[:, dt_:dt_ + 1])
            # V columns for heads [8*half, 8*half+8)
            with tc.tile_pool(name="vproj", bufs=1) as vp, \
                    tc.tile_pool(name="vxt", bufs=2) as vxt:
                wv_sb = vp.tile([128, MC, 512], f32r, tag="wv")
                nc.sync.dma_start(
                    out=wv_sb, in_=wvre[:, :, half * 512:(half + 1) * 512])
                for kt in range(NKT):
                    xt_sb = vxt.tile([128, MC, 128], f32r, tag="xtv")
                    nc.sync.dma_start(
                        out=xt_sb, in_=xre[:, :, kt * 128:(kt + 1) * 128])
                    ps = proj_ps.tile([128, 512], f32, tag="ps")
                    for m in range(MC):
                        nc.tensor.matmul(
                            ps[:], xt_sb[:, m, :], wv_sb[:, m, :],
                            start=(m == 0), stop=(m == MC - 1))
                    nc.vector.tensor_copy(
                        V_sb[:, kt, half * 8:(half + 1) * 8, 0:HD],
                        ps[:].rearrange("p (h d) -> p h d", d=HD))
                    nc.vector.tensor_scalar(
                        out=V_sb[:, kt, half * 8:(half + 1) * 8, HD:HD + 1],
                        in0=ps[:].rearrange("p (h d) -> p h d", d=HD)[:, :, 0:1],
                        scalar1=0.0, scalar2=1.0, op0=MULT, op1=ADD)
            # attention for this half's heads (overlaps next half's K/V proj)
            for h0 in range(8 * half, 8 * half + 8, 2):
                attention(h0, NKT)

        # ---- output projection: YT[n, jq] ----
        with tc.tile_pool(name="oproj", bufs=2) as op:
            for q4 in range(4):
                wo_sb = op.tile([128, MC, 256], f32r, tag="wo")
                nc.sync.dma_start(
                    out=wo_sb,
                    in_=woT.rearrange("(c p) n -> p c n", p=128)[:, :, q4 * 256:(q4 + 1) * 256])
                for nt_i in range(2):
                    nt = q4 * 2 + nt_i
                    ps = proj_ps.tile([128, JQ], f32, tag="ps")
                    for c in range(MC):
                        nc.tensor.matmul(
                            ps[:], wo_sb[:, c, nt_i * 128:(nt_i + 1) * 128],
                            AT_sb[:, c, :],
                            start=(c == 0), stop=(c == MC - 1))
                    yt = yt_p.tile([128, JQ], f32, tag="yt")
                    nc.scalar.activation(out=yt[:], in_=ps[:], func=IDENT,
                                         bias=ob_sb[:, nt:nt + 1])
                    nc.sync.dma_start(out=out[nt * 128:(nt + 1) * 128, :], in_=yt[:])

    _split_waits(nc, mybir)
    return nc


def _get_nc():
    if "nc" not in _CACHE:
        _CACHE["nc"] = _build()
    return _CACHE["nc"]


def _make_inputs(x, mask, Wq, bq, Wk, bk, Wv, bv, Wo, bo):
    f = np.float32
    x = np.asarray(x, f)
    mask = np.asarray(mask)
    Wq, bq = np.asarray(Wq, f), np.asarray(bq, f)
    Wk, bk = np.asarray(Wk, f), np.asarray(bk, f)
    Wv, bv = np.asarray(Wv, f), np.asarray(bv, f)
    Wo, bo = np.asarray(Wo, f), np.asarray(bo, f)

    wqT = np.ascontiguousarray(Wq.T)
    wkT = np.ascontiguousarray(Wk.T)
    wvT = np.ascontiguousarray(Wv.T)
    woT = np.ascontiguousarray(Wo.T)
    bq8 = (bq / 8.0).astype(f)
    obias = (bo + Wo @ bv).astype(f)

    xTb = [np.ascontiguousarray(x[b].T) for b in range(B)]
    pmbb = [((mask[b].astype(f) - 1.0) * 1e4).astype(f) for b in range(B)]

    ii, jj = np.meshgrid(np.arange(128), np.arange(JB_N), indexing="ij")
    onesc = np.ones((1, HD), f)

    ins = []
    for c in range(N_CORES):
        b, r = c // 4, c % 4
        cm = np.empty((8, 128, JB_N), f)
        for tp in range(8):
            cm[tp] = (128 * tp + ii <= 4 * jj + r).astype(f)
        ins.append({
            "xT": xTb[b],
            "xqT": np.ascontiguousarray(x[b].T[:, r::4]),
            "wqT": wqT, "wkT": wkT, "wvT": wvT, "woT": woT,
            "bq8": bq8, "bk": bk, "obias": obias,
            "pmb": pmbb[b],
            "cmask": cm,
            "onesc": onesc,
        })
    return ins


def _run(ins, trace=False):
    from concourse.bass_utils import run_bass_kernel_spmd
    nc = _get_nc()
    return run_bass_kernel_spmd(nc, ins, list(range(N_CORES)), trace=trace)


def kernel(x, mask, Wq, bq, Wk, bk, Wv, bv, Wo, bo):
    ins = _make_inputs(x, mask, Wq, bq, Wk, bk, Wv, bv, Wo, bo)
    res = _run(ins)
    out = np.empty((B, S, D), np.float32)
    for c in range(N_CORES):
        b, r = c // 4, c % 4
        out[b, r::4, :] = res.results[c]["o"].T
    return out



# revision 14
# speedup vs baseline: 1.9937x; 1.9937x over previous
"""Multi-head self-attention (B=2, S=2048, D=1024, H=16, causal+padding mask)
on 8 Trainium2 NeuronCores via Bass/Tile, SPMD.

Sharding: core c -> batch b = c//4, head group hg = c%4 (heads 4hg..4hg+3,
i.e. a 256-wide slice of the model dim). Each core computes Q/K/V projections
only for its slice (no duplicated K/V work), blocked-causal attention for its
4 heads over all 2048 queries, and a row-parallel partial O-projection
Y_c = AT_c^T Wo_slice. The host sums the 4 partials per batch and adds the
output bias. Algebraic simplifications:
  - K bias is dropped: score(q,k) = Q_q.(xWk + bk)_k adds Q_q.bk, constant
    over k for fixed q, which softmax cancels.
  - V bias folds out: softmax rows sum to 1, so its contribution is
    bv @ Wo^T, a constant added on the host together with bo.
  - The 1/sqrt(64) score scale is folded into Wq/bq on the host.

Dataflow (per core, all transposed so no on-chip transposes are needed):
  x^T[d, s]     loaded once in 4 seq chunks of 512
  QT[dh, q]     = (Wq_sl x^T)*0.125 + bq*0.125   (chains of 8 matmuls, 512-free)
  KT[dh, k]     = Wk_sl x^T                       (no bias)
  V [k, dh+1]   = x Wv_sl^T with a ones column   (col 64 = softmax denominator)
  ST[k, q]      = KT_h^T QT_h per (head, 128-key tile, 512-query block)
  E             = exp(ST + padmask_bias); diagonal tiles *= causal01 (gpsimd)
  OT[dh+1, q]   += V_aug^T E                      (row 64 = denominators r)
  AT[dh, q]     = OT * (1/r)  (reciprocal of row 64, matmul-broadcast, DVE mul)
  Y^T[n, q]     = Wo_sl^T AT  partial, summed across cores on the host
Matmuls in float32r (full-rate fp32, ~1e-4 rel err). Softmax skips
max-subtraction: |scores| < ~5 so exp is safe; padding-masked keys get -1e4
added pre-exp which underflows to 0.

Schedule: attention for query block j is interleaved at (head-pair, key-tile)
granularity with the projection chains of seq chunk j+1 and the O-projection
of block j-1, keeping the PE stream continuous (the Act engine's exp
throughput is slightly below the PE's ST+PV rate, so pure attention would
stall the PE and drop it out of its high clock p-state).
"""

import sys

if "/opt/trn_rl_repo" not in sys.path:
    sys.path.insert(0, "/opt/trn_rl_repo")

import numpy as np

B, S, D, H, HD = 2, 2048, 1024, 16, 64
N_CORES = 8
HPC = 4              # heads per core
DS = HPC * HD        # 256: model-dim slice per core
QB = 512             # query block (free dim of ST/PV/proj matmuls)
NQB = S // QB        # 4
MC = D // 128        # 8 contraction chunks
NKT = S // 128       # 16 key tiles

_CACHE = {}


def _split_waits(nc, mybir):
    """This walrus build accepts only one sync-wait per instruction; move
    extra waits onto NOPs inserted just before, on the same engine."""
    n_new = 0
    for f in nc.m.functions:
        for blk in f.blocks:
            out = []
            for inst in blk.instructions:
                si = inst.sync_info
                if si is not None and si.on_wait is not None and len(si.on_wait) > 1:
                    waits = list(si.on_wait)
                    for w in waits[:-1]:
                        n_new += 1
                        out.append(mybir.InstNoOp(
                            name=f"I-waitsplit-{n_new}",
                            engine=inst.engine,
                            ins=[], outs=[],
                            sync_info=mybir.SyncInfo(on_wait=[w], on_update=[]),
                        ))
                    inst.sync_info = mybir.SyncInfo(
                        on_wait=[waits[-1]], on_update=list(si.on_update or []))
                out.append(inst)
            blk.instructions[:] = out
    return n_new


def _build():
    import concourse.bass as bass
    import concourse.mybir as mybir
    import concourse.tile as tile
    from contextlib import ExitStack

    f32 = mybir.dt.float32
    f32r = mybir.dt.float32r
    EXP = mybir.ActivationFunctionType.Exp
    MULT = mybir.AluOpType.mult
    ADD = mybir.AluOpType.add

    nc = bass.Bass()
    xT = nc.declare_dram_parameter("xT", [D, S], f32r, isOutput=False)
    wq = nc.declare_dram_parameter("wq", [D, DS], f32r, isOutput=False)
    wk = nc.declare_dram_parameter("wk", [D, DS], f32r, isOutput=False)
    wv = nc.declare_dram_parameter("wv", [D, DS], f32r, isOutput=False)
    wo = nc.declare_dram_parameter("wo", [DS, D], f32r, isOutput=False)
    bq = nc.declare_dram_parameter("bq", [DS], f32, isOutput=False)
    pmb = nc.declare_dram_parameter("pmb", [S], f32, isOutput=False)
    cmask = nc.declare_dram_parameter("cmask", [4, 128, QB], f32r, isOutput=False)
    onesc = nc.declare_dram_parameter("onesc", [1, HD], f32r, isOutput=False)
    out = nc.declare_dram_parameter("o", [D, S], f32, isOutput=True)

    with tile.TileContext(nc) as tc, ExitStack() as ctx, \
            nc.allow_low_precision("fp32r matmul inputs keep ~19 bits"):
        ec = ctx.enter_context
        consts = ec(tc.tile_pool(name="consts", bufs=1))
        big = ec(tc.tile_pool(name="big", bufs=1))
        e_p = ec(tc.tile_pool(name="e", bufs=6))
        rcp_p = ec(tc.tile_pool(name="rcp", bufs=2))
        yt_p = ec(tc.tile_pool(name="yt", bufs=2))
        proj_ps = ec(tc.tile_pool(name="proj_ps", bufs=2, space="PSUM"))
        st_ps = ec(tc.tile_pool(name="st_ps", bufs=3, space="PSUM"))
        ot_ps = ec(tc.tile_pool(name="ot_ps", bufs=2, space="PSUM"))

        # ---- constants into SBUF ----
        wq_sb = consts.tile([128, MC, DS], f32r, tag="wq")
        nc.sync.dma_start(out=wq_sb, in_=wq.rearrange("(c p) n -> p c n", p=128))
        bq_sb = consts.tile([128, 2], f32, tag="bq")
        nc.sync.dma_start(out=bq_sb, in_=bq.rearrange("(c p) -> p c", p=128))
        pmb_sb = consts.tile([128, NKT], f32, tag="pmb")
        nc.sync.dma_start(out=pmb_sb, in_=pmb.rearrange("(t p) -> p t", p=128))

        xre = xT.rearrange("(c p) k -> p c k", p=128)
        x_sb = [big.tile([128, MC, QB], f32r, tag=f"x{s}", name=f"x{s}")
                for s in range(NQB)]
        nc.sync.dma_start(out=x_sb[0], in_=xre[:, :, 0:QB])

        wk_sb = consts.tile([128, MC, DS], f32r, tag="wk")
        nc.sync.dma_start(out=wk_sb, in_=wk.rearrange("(c p) n -> p c n", p=128))
        wv_sb = consts.tile([128, MC, DS], f32r, tag="wv")
        nc.sync.dma_start(out=wv_sb, in_=wv.rearrange("(c p) n -> p c n", p=128))
        cm_sb = consts.tile([128, 4, QB], f32r, tag="cm")
        nc.sync.dma_start(out=cm_sb, in_=cmask.rearrange("t p j -> p t j"))
        nc.sync.dma_start(out=x_sb[1], in_=xre[:, :, QB:2 * QB])
        wo_sb = consts.tile([128, 2, D], f32r, tag="wo")
        nc.sync.dma_start(out=wo_sb, in_=wo.rearrange("(c p) n -> p c n", p=128))
        nc.sync.dma_start(out=x_sb[2], in_=xre[:, :, 2 * QB:3 * QB])
        nc.sync.dma_start(out=x_sb[3], in_=xre[:, :, 3 * QB:4 * QB])

        ones_sb = consts.tile([1, HD], f32r, tag="ones")
        nc.sync.dma_start(out=ones_sb, in_=onesc[:, :])

        # persistent activations
        QT_sb = big.tile([128, 2, S], f32r, tag="qt")        # 16KB/part
        KT_sb = big.tile([128, 2, S], f32r, tag="kt")        # 16KB/part
        V_sb = big.tile([128, NKT, HPC, HD + 1], f32r, tag="v")  # 16.6KB/part
        AT_sb = big.tile([128, 2, S], f32r, tag="at")        # 16KB/part

        def proj_chunk_chains(s):
            """Return emit-closures, one per PSUM chain, for Q/K/V projection
            of seq chunk s (queries/keys [512s, 512s+512))."""
            chains = []
            for t in range(2):
                def qchain(t=t):
                    ps = proj_ps.tile([128, QB], f32, tag="ps")
                    for m in range(MC):
                        nc.tensor.matmul(
                            ps[:], wq_sb[:, m, t * 128:(t + 1) * 128],
                            x_sb[s][:, m, :], start=(m == 0), stop=(m == MC - 1))
                    nc.vector.tensor_scalar_add(
                        out=QT_sb[:, t, s * QB:(s + 1) * QB], in0=ps[:],
                        scalar1=bq_sb[:, t:t + 1])
                chains.append(qchain)
            for t in range(2):
                def kchain(t=t):
                    ps = proj_ps.tile([128, QB], f32, tag="ps")
                    for m in range(MC):
                        nc.tensor.matmul(
                            ps[:], wk_sb[:, m, t * 128:(t + 1) * 128],
                            x_sb[s][:, m, :], start=(m == 0), stop=(m == MC - 1))
                    nc.vector.tensor_copy(
                        KT_sb[:, t, s * QB:(s + 1) * QB], ps[:])
                chains.append(kchain)
            for ktl in range(4):
                def vchain(ktl=ktl):
                    kt = 4 * s + ktl
                    ps = proj_ps.tile([128, QB], f32, tag="ps")
                    for m in range(MC):
                        nc.tensor.matmul(
                            ps[:, 0:DS], x_sb[s][:, m, ktl * 128:(ktl + 1) * 128],
                            wv_sb[:, m, :], start=(m == 0), stop=(m == MC - 1))
                    nc.vector.tensor_copy(
                        V_sb[:, kt, :, 0:HD],
                        ps[:, 0:DS].rearrange("p (h d) -> p h d", d=HD))
                    nc.vector.tensor_scalar(
                        out=V_sb[:, kt, :, HD:HD + 1],
                        in0=ps[:, 0:DS].rearrange(
                            "p (h d) -> p h d", d=HD)[:, :, 0:1],
                        scalar1=0.0, scalar2=1.0, op0=MULT, op1=ADD)
                chains.append(vchain)
            return chains

        def oproj_chains(j):
            """Partial output projection for query block j (AT must be done)."""
            chains = []
            for nt in range(MC):
                def ochain(nt=nt):
                    ps = proj_ps.tile([128, QB], f32, tag="ps")
                    for c in range(2):
                        nc.tensor.matmul(
                            ps[:], wo_sb[:, c, nt * 128:(nt + 1) * 128],
                            AT_sb[:, c, j * QB:(j + 1) * QB],
                            start=(c == 0), stop=(c == 1))
                    yt = yt_p.tile([128, QB], f32, tag="yt")
                    nc.vector.tensor_copy(yt[:], ps[:])
                    nc.sync.dma_start(
                        out=out[nt * 128:(nt + 1) * 128, j * QB:(j + 1) * QB],
                        in_=yt[:])
                chains.append(ochain)
            return chains

        def attention_block(j, fillers):
            """Attention for query block j (all 4 heads, processed as 2 pairs),
            with filler emit-closures spread between (pair, key-tile) groups to
            keep the PE busy while exp drains."""
            nkt = 4 * (j + 1)
            groups = [(p, kt) for p in range(2) for kt in range(nkt)]
            nfill = len(fillers)
            fi = 0
            ots = {}
            for gi, (p, kt) in enumerate(groups):
                if kt == 0:
                    ots[p] = [ot_ps.tile([HD + 1, QB], f32, tag="ot",
                                         name=f"ot{j}_{p}_{hi2}")
                              for hi2 in range(2)]
                for hi in range(2):
                    h = 2 * p + hi
                    pr, hw = h // 2, 64 * (h % 2)
                    st = st_ps.tile([128, QB], f32, tag="st")
                    nc.tensor.matmul(
                        st[:],
                        KT_sb[hw:hw + 64, pr, kt * 128:(kt + 1) * 128],
                        QT_sb[hw:hw + 64, pr, j * QB:(j + 1) * QB],
                        start=True, stop=True)
                    e = e_p.tile([128, QB], f32r, tag="e")
                    nc.scalar.activation(out=e[:], in_=st[:], func=EXP,
                                         bias=pmb_sb[:, kt:kt + 1])
                    tp = kt - 4 * j
                    if tp >= 0:
                        nc.gpsimd.tensor_mul(e[:], e[:], cm_sb[:, tp, :])
                    nc.tensor.matmul(ots[p][hi][:], V_sb[:, kt, h, :], e[:],
                                     start=(kt == 0), stop=(kt == nkt - 1))
                # spread fillers evenly across groups
                want = (gi + 1) * nfill // len(groups)
                while fi < want:
                    fillers[fi]()
                    fi += 1
                if kt == nkt - 1:
                    for hi in range(2):
                        h = 2 * p + hi
                        pr, hw = h // 2, 64 * (h % 2)
                        otp = ots[p][hi]
                        rcp = rcp_p.tile([1, QB], f32r, tag="rcp")
                        nc.vector.reciprocal(out=rcp[:], in_=otp[HD:HD + 1, :])
                        bc = st_ps.tile([128, QB], f32, tag="st")
                        nc.tensor.matmul(bc[0:HD, :], ones_sb[:], rcp[:],
                                         start=True, stop=True)
                        rb = rcp_p.tile([HD, QB], f32r, tag="rb")
                        nc.vector.tensor_copy(rb[:], bc[0:HD, :])
                        nc.vector.tensor_mul(
                            AT_sb[hw:hw + 64, pr, j * QB:(j + 1) * QB],
                            otp[0:HD, :], rb[:])
            while fi < nfill:
                fillers[fi]()
                fi += 1

        # ---- schedule ----
        for ch in proj_chunk_chains(0):
            ch()
        attention_block(0, proj_chunk_chains(1))
        attention_block(1, proj_chunk_chains(2) + oproj_chains(0))
        attention_block(2, proj_chunk_chains(3) + oproj_chains(1))
        attention_block(3, oproj_chains(2))
        for ch in oproj_chains(3):
            ch()

    _split_waits(nc, mybir)
    return nc


def _get_nc():
    if "nc" not in _CACHE:
        _CACHE["nc"] = _build()
    return _CACHE["nc"]


def _make_inputs(x, mask, Wq, bq, Wk, bk, Wv, bv, Wo, bo):
    f = np.float32
    x = np.asarray(x, f)
    mask = np.asarray(mask)
    Wq, bq = np.asarray(Wq, f), np.asarray(bq, f)
    Wk = np.asarray(Wk, f)
    Wv = np.asarray(Wv, f)
    Wo = np.asarray(Wo, f)

    wqT = np.ascontiguousarray(Wq.T) * np.float32(0.125)
    wkT = np.ascontiguousarray(Wk.T)
    wvT = np.ascontiguousarray(Wv.T)
    woT = np.ascontiguousarray(Wo.T)
    bq8 = (bq * 0.125).astype(f)

    xTb = [np.ascontiguousarray(x[b].T) for b in range(B)]
    pmbb = [((mask[b].astype(f) - 1.0) * 1e4).astype(f) for b in range(B)]

    pp, jj = np.meshgrid(np.arange(128), np.arange(QB), indexing="ij")
    cm = np.empty((4, 128, QB), f)
    for tp in range(4):
        cm[tp] = (128 * tp + pp <= jj).astype(f)

    ins = []
    for c in range(N_CORES):
        b, hg = c // 4, c % 4
        sl = slice(DS * hg, DS * (hg + 1))
        ins.append({
            "xT": xTb[b],
            "wq": np.ascontiguousarray(wqT[:, sl]),
            "wk": np.ascontiguousarray(wkT[:, sl]),
            "wv": np.ascontiguousarray(wvT[:, sl]),
            "wo": np.ascontiguousarray(woT[sl, :]),
            "bq": np.ascontiguousarray(bq8[sl]),
            "pmb": pmbb[b],
            "cmask": cm,
            "onesc": np.ones((1, HD), f),
        })
    return ins


def _run(ins, trace=False):
    from concourse.bass_utils import run_bass_kernel_spmd
    nc = _get_nc()
    return run_bass_kernel_spmd(nc, ins, list(range(N_CORES)), trace=trace)


def kernel(x, mask, Wq, bq, Wk, bk, Wv, bv, Wo, bo):
    ins = _make_inputs(x, mask, Wq, bq, Wk, bk, Wv, bv, Wo, bo)
    res = _run(ins)
    obias = (np.asarray(bo, np.float32)
             + np.asarray(Wo, np.float32) @ np.asarray(bv, np.float32))
    out = np.empty((B, S, D), np.float32)
    for b in range(B):
        acc = res.results[4 * b]["o"].astype(np.float32)
        for hg in range(1, 4):
            acc = acc + res.results[4 * b + hg]["o"]
        out[b] = acc.T + obias
    return out


# revision 26
# speedup vs baseline: 2.3724x; 1.1899x over previous
"""Multi-head self-attention (B=2, S=2048, D=1024, H=16, causal+padding mask)
on 8 Trainium2 NeuronCores via Bass/Tile, SPMD.

Sharding: core c -> batch b = c//4, head group hg = c%4 (heads 4hg..4hg+3,
i.e. a 256-wide slice of the model dim). Each core computes Q/K/V projections
only for its slice (no duplicated K/V work), blocked-causal attention for its
4 heads over all 2048 queries, and a row-parallel partial O-projection
Y_c = AT_c^T Wo_slice. The host sums the 4 partials per batch and adds the
output bias. Algebraic simplifications:
  - K bias is dropped: score(q,k) = Q_q.(xWk + bk)_k adds Q_q.bk, constant
    over k for fixed q, which softmax cancels.
  - V bias folds out: softmax rows sum to 1, so its contribution is
    bv @ Wo^T, a constant added on the host together with bo.
  - The 1/sqrt(64) score scale is folded into Wq/bq on the host.

Dataflow (per core, all transposed so no on-chip transposes are needed):
  x^T[d, s]     loaded once in 4 seq chunks of 512
  QT[dh, q]     = (Wq_sl x^T)*0.125 + bq*0.125   (chains of 8 matmuls, 512-free)
  KT[dh, k]     = Wk_sl x^T                       (no bias)
  V [k, dh+1]   = x Wv_sl^T with a ones column   (col 64 = softmax denominator)
  ST[k, q]      = KT_h^T QT_h per (head, 128-key tile, 512-query block)
  E             = exp(ST + padmask_bias); diagonal tiles *= causal01 (gpsimd)
  OT[dh+1, q]   += V_aug^T E                      (row 64 = denominators r)
  AT[dh, q]     = OT * (1/r)  (reciprocal of row 64, matmul-broadcast, DVE mul)
  Y^T[n, q]     = Wo_sl^T AT  partial, summed across cores on the host
Matmuls in float32r (full-rate fp32, ~1e-4 rel err). Softmax skips
max-subtraction: |scores| < ~5 so exp is safe; padding-masked keys get -1e4
added pre-exp which underflows to 0.

Schedule: attention for query block j is interleaved at (head-pair, key-tile)
granularity with the projection chains of seq chunk j+1 and the O-projection
of block j-1, keeping the PE stream continuous (the Act engine's exp
throughput is slightly below the PE's ST+PV rate, so pure attention would
stall the PE and drop it out of its high clock p-state).
"""

import sys

if "/opt/trn_rl_repo" not in sys.path:
    sys.path.insert(0, "/opt/trn_rl_repo")

import numpy as np

B, S, D, H, HD = 2, 2048, 1024, 16, 64
N_CORES = 8
HPC = 4              # heads per core
DS = HPC * HD        # 256: model-dim slice per core
QB = 512             # query block (free dim of ST/PV/proj matmuls)
NQB = S // QB        # 4
MC = D // 128        # 8 contraction chunks
NKT = S // 128       # 16 key tiles

_CACHE = {}


def _split_waits(nc, mybir):
    """This walrus build accepts only one sync-wait per instruction; move
    extra waits onto NOPs inserted just before, on the same engine."""
    n_new = 0
    for f in nc.m.functions:
        for blk in f.blocks:
            out = []
            for inst in blk.instructions:
                si = inst.sync_info
                if si is not None and si.on_wait is not None and len(si.on_wait) > 1:
                    waits = list(si.on_wait)
                    for w in waits[:-1]:
                        n_new += 1
                        out.append(mybir.InstNoOp(
                            name=f"I-waitsplit-{n_new}",
                            engine=inst.engine,
                            ins=[], outs=[],
                            sync_info=mybir.SyncInfo(on_wait=[w], on_update=[]),
                        ))
                    inst.sync_info = mybir.SyncInfo(
                        on_wait=[waits[-1]], on_update=list(si.on_update or []))
                out.append(inst)
            blk.instructions[:] = out
    return n_new


def _build():
    import concourse.bass as bass
    import concourse.mybir as mybir
    import concourse.tile as tile
    from contextlib import ExitStack

    f32 = mybir.dt.float32
    f32r = mybir.dt.float32r
    EXP = mybir.ActivationFunctionType.Exp
    COPY = mybir.ActivationFunctionType.Copy
    MULT = mybir.AluOpType.mult
    ADD = mybir.AluOpType.add

    nc = bass.Bass()
    xT = nc.declare_dram_parameter("xT", [D, S], f32r, isOutput=False)
    wq = nc.declare_dram_parameter("wq", [D, DS], f32r, isOutput=False)
    wk = nc.declare_dram_parameter("wk", [D, DS], f32r, isOutput=False)
    wv = nc.declare_dram_parameter("wv", [D, DS], f32r, isOutput=False)
    wo = nc.declare_dram_parameter("wo", [DS, D], f32r, isOutput=False)
    bq = nc.declare_dram_parameter("bq", [DS], f32, isOutput=False)
    pmb = nc.declare_dram_parameter("pmb", [S], f32, isOutput=False)
    cmask = nc.declare_dram_parameter("cmask", [128, 384], f32r, isOutput=False)
    onesc = nc.declare_dram_parameter("onesc", [1, HD], f32r, isOutput=False)
    out = nc.declare_dram_parameter("o", [D, S], f32, isOutput=True)

    with tile.TileContext(nc) as tc, ExitStack() as ctx, \
            nc.allow_low_precision("fp32r matmul inputs keep ~19 bits"):
        ec = ctx.enter_context
        consts = ec(tc.tile_pool(name="consts", bufs=1))
        big = ec(tc.tile_pool(name="big", bufs=1))
        e_p = ec(tc.tile_pool(name="e", bufs=6))
        rcp_p = ec(tc.tile_pool(name="rcp", bufs=2))
        yt_p = ec(tc.tile_pool(name="yt", bufs=2))
        proj_ps = ec(tc.tile_pool(name="proj_ps", bufs=2, space="PSUM"))
        st_ps = ec(tc.tile_pool(name="st_ps", bufs=4, space="PSUM"))
        ot_ps = ec(tc.tile_pool(name="ot_ps", bufs=2, space="PSUM"))

        # ---- constants into SBUF ----
        wq_sb = consts.tile([128, MC, DS], f32r, tag="wq")
        nc.sync.dma_start(out=wq_sb, in_=wq.rearrange("(c p) n -> p c n", p=128))
        bq_sb = consts.tile([128, 2], f32, tag="bq")
        nc.sync.dma_start(out=bq_sb, in_=bq.rearrange("(c p) -> p c", p=128))
        pmb_sb = consts.tile([128, NKT], f32, tag="pmb")
        nc.sync.dma_start(out=pmb_sb, in_=pmb.rearrange("(t p) -> p t", p=128))

        xre = xT.rearrange("(c p) k -> p c k", p=128)
        x_sb = [big.tile([128, MC, QB], f32r, tag=f"x{s}", name=f"x{s}")
                for s in range(NQB)]
        nc.sync.dma_start(out=x_sb[0], in_=xre[:, :, 0:QB])

        wk_sb = consts.tile([128, MC, DS], f32r, tag="wk")
        nc.sync.dma_start(out=wk_sb, in_=wk.rearrange("(c p) n -> p c n", p=128))
        wv_sb = consts.tile([128, MC, DS], f32r, tag="wv")
        nc.sync.dma_start(out=wv_sb, in_=wv.rearrange("(c p) n -> p c n", p=128))
        cm_sb = consts.tile([128, 384], f32r, tag="cm")
        nc.sync.dma_start(out=cm_sb, in_=cmask[:, :])
        nc.sync.dma_start(out=x_sb[1], in_=xre[:, :, QB:2 * QB])
        wo_sb = consts.tile([128, 2, D], f32r, tag="wo")
        nc.sync.dma_start(out=wo_sb, in_=wo.rearrange("(c p) n -> p c n", p=128))
        nc.sync.dma_start(out=x_sb[2], in_=xre[:, :, 2 * QB:3 * QB])
        nc.sync.dma_start(out=x_sb[3], in_=xre[:, :, 3 * QB:4 * QB])

        ones_sb = consts.tile([1, HD], f32r, tag="ones")
        nc.sync.dma_start(out=ones_sb, in_=onesc[:, :])

        # persistent activations
        QT_sb = big.tile([128, 2, S], f32r, tag="qt")        # 16KB/part
        KT_sb = big.tile([128, 2, S], f32r, tag="kt")        # 16KB/part
        V_sb = big.tile([128, NKT, HPC, HD + 1], f32r, tag="v")  # 16.6KB/part
        AT_sb = big.tile([128, 2, S], f32r, tag="at")        # 16KB/part

        def proj_chunk_chains(s):
            """Return emit-closures, one per PSUM chain, for Q/K/V projection
            of seq chunk s (queries/keys [512s, 512s+512))."""
            chains = []
            for t in range(2):
                def qchain(t=t):
                    ps = proj_ps.tile([128, QB], f32, tag="ps")
                    for m in range(MC):
                        nc.tensor.matmul(
                            ps[:], wq_sb[:, m, t * 128:(t + 1) * 128],
                            x_sb[s][:, m, :], start=(m == 0), stop=(m == MC - 1))
                    nc.vector.tensor_scalar_add(
                        out=QT_sb[:, t, s * QB:(s + 1) * QB], in0=ps[:],
                        scalar1=bq_sb[:, t:t + 1])
                chains.append(qchain)
            for t in range(2):
                def kchain(t=t):
                    ps = proj_ps.tile([128, QB], f32, tag="ps")
                    for m in range(MC):
                        nc.tensor.matmul(
                            ps[:], wk_sb[:, m, t * 128:(t + 1) * 128],
                            x_sb[s][:, m, :], start=(m == 0), stop=(m == MC - 1))
                    nc.vector.tensor_copy(
                        KT_sb[:, t, s * QB:(s + 1) * QB], ps[:])
                chains.append(kchain)
            for ktl in range(4):
                def vchain(ktl=ktl):
                    kt = 4 * s + ktl
                    ps = proj_ps.tile([128, QB], f32, tag="ps")
                    for m in range(MC):
                        nc.tensor.matmul(
                            ps[:, 0:DS], x_sb[s][:, m, ktl * 128:(ktl + 1) * 128],
                            wv_sb[:, m, :], start=(m == 0), stop=(m == MC - 1))
                    nc.vector.tensor_copy(
                        V_sb[:, kt, :, 0:HD],
                        ps[:, 0:DS].rearrange("p (h d) -> p h d", d=HD))
                    nc.vector.tensor_scalar(
                        out=V_sb[:, kt, :, HD:HD + 1],
                        in0=ps[:, 0:DS].rearrange(
                            "p (h d) -> p h d", d=HD)[:, :, 0:1],
                        scalar1=0.0, scalar2=1.0, op0=MULT, op1=ADD)
                chains.append(vchain)
            return chains

        def oproj_chains(j):
            """Partial output projection for query block j (AT must be done)."""
            chains = []
            for nt in range(MC):
                def ochain(nt=nt):
                    ps = proj_ps.tile([128, QB], f32, tag="ps")
                    for c in range(2):
                        nc.tensor.matmul(
                            ps[:], wo_sb[:, c, nt * 128:(nt + 1) * 128],
                            AT_sb[:, c, j * QB:(j + 1) * QB],
                            start=(c == 0), stop=(c == 1))
                    yt = yt_p.tile([128, QB], f32, tag="yt")
                    nc.vector.tensor_copy(yt[:], ps[:])
                    nc.sync.dma_start(
                        out=out[nt * 128:(nt + 1) * 128, j * QB:(j + 1) * QB],
                        in_=yt[:])
                chains.append(ochain)
            return chains

        def attention_block(j, fillers):
            """Attention for query block j (all 4 heads, processed as 2 pairs).

            Software-pipelined one key-tile ahead: the PV pair for tile kt is
            emitted after the ST/exp pair for tile kt+1, so exp latency hides
            behind other PE work. Filler emit-closures (projection/O-proj
            chains) are spread between steps to cover the Act engine's lower
            throughput. Diagonal tiles only compute/exp/mask the causally
            reachable column range [c0, 512): columns below c0 see none of the
            tile's keys, and the mask multiply only covers the partial window.
            """
            nkt = 4 * (j + 1)
            nfill = len(fillers)
            fi = 0
            nsteps = 2 * (nkt + 1)
            si = 0
            es = {}

            def c0_of(kt):
                tp = kt - 4 * j
                return 0 if tp < 0 else min(128 * tp, 256)

            def st_exp(p, kt):
                c0 = c0_of(kt)
                tp = kt - 4 * j
                for hi in range(2):
                    h = 2 * p + hi
                    pr, hw = h // 2, 64 * (h % 2)
                    st = st_ps.tile([128, QB], f32, tag="st")
                    nc.tensor.matmul(
                        st[:, c0:],
                        KT_sb[hw:hw + 64, pr, kt * 128:(kt + 1) * 128],
                        QT_sb[hw:hw + 64, pr, j * QB + c0:(j + 1) * QB],
                        start=True, stop=True)
                    e = e_p.tile([128, QB], f32r, tag="e")
                    nc.scalar.activation(out=e[:, c0:], in_=st[:, c0:],
                                         func=EXP, bias=pmb_sb[:, kt:kt + 1])
                    if tp >= 0:
                        if tp < 3:
                            nc.gpsimd.tensor_mul(
                                e[:, 128 * tp:128 * (tp + 1)],
                                e[:, 128 * tp:128 * (tp + 1)],
                                cm_sb[:, 0:128])
                        else:
                            nc.gpsimd.tensor_mul(
                                e[:, 256:512], e[:, 256:512],
                                cm_sb[:, 128:384])
                    es[(p, kt, hi)] = e

            def pv(p, kt):
                c0 = c0_of(kt)
                for hi in range(2):
                    h = 2 * p + hi
                    nc.tensor.matmul(
                        ots[p][hi][:, c0:], V_sb[:, kt, h, :],
                        es.pop((p, kt, hi))[:, c0:],
                        start=(kt == 0), stop=(kt == nkt - 1))

            ots = {}
            for p in range(2):
                ots[p] = [ot_ps.tile([HD + 1, QB], f32, tag="ot",
                                     name=f"ot{j}_{p}_{hi2}")
                          for hi2 in range(2)]
                for kt in range(nkt):
                    st_exp(p, kt)
                    if kt >= 1:
                        pv(p, kt - 1)
                    si += 1
                    want = si * nfill // nsteps
                    while fi < want:
                        fillers[fi]()
                        fi += 1
                pv(p, nkt - 1)
                si += 1
                for hi in range(2):
                    h = 2 * p + hi
                    pr, hw = h // 2, 64 * (h % 2)
                    otp = ots[p][hi]
                    dn = rcp_p.tile([1, QB], f32, tag="dn")
                    nc.vector.tensor_copy(dn[:], otp[HD:HD + 1, :])
                    rcp = rcp_p.tile([1, QB], f32, tag="rcp")
                    nc.vector.reciprocal_approx_fast(out=rcp[:], in_=dn[:])
                    rcpr = rcp_p.tile([1, QB], f32r, tag="rcpr")
                    nc.vector.tensor_copy(rcpr[:], rcp[:])
                    bc = st_ps.tile([128, QB], f32, tag="st")
                    nc.tensor.matmul(bc[0:HD, :], ones_sb[:], rcpr[:],
                                     start=True, stop=True)
                    rb = rcp_p.tile([HD, QB], f32r, tag="rb")
                    nc.scalar.activation(out=rb[:], in_=bc[0:HD, :],
                                         func=COPY)
                    nc.vector.tensor_mul(
                        AT_sb[hw:hw + 64, pr, j * QB:(j + 1) * QB],
                        otp[0:HD, :], rb[:])
            while fi < nfill:
                fillers[fi]()
                fi += 1

        # ---- schedule ----
        for ch in proj_chunk_chains(0):
            ch()
        attention_block(0, proj_chunk_chains(1))
        attention_block(1, proj_chunk_chains(2))
        attention_block(2, proj_chunk_chains(3) + oproj_chains(0))
        attention_block(3, oproj_chains(1) + oproj_chains(2))
        for ch in oproj_chains(3):
            ch()

    from concourse.library_overlay import lower_extended_insts
    lower_extended_insts(nc)
    _split_waits(nc, mybir)
    return nc


def _get_nc():
    if "nc" not in _CACHE:
        _CACHE["nc"] = _build()
    return _CACHE["nc"]


def _make_inputs(x, mask, Wq, bq, Wk, bk, Wv, bv, Wo, bo):
    f = np.float32
    x = np.asarray(x, f)
    mask = np.asarray(mask)
    Wq, bq = np.asarray(Wq, f), np.asarray(bq, f)
    Wk = np.asarray(Wk, f)
    Wv = np.asarray(Wv, f)
    Wo = np.asarray(Wo, f)

    wqT = np.ascontiguousarray(Wq.T) * np.float32(0.125)
    wkT = np.ascontiguousarray(Wk.T)
    wvT = np.ascontiguousarray(Wv.T)
    woT = np.ascontiguousarray(Wo.T)
    bq8 = (bq * 0.125).astype(f)

    xTb = [np.ascontiguousarray(x[b].T) for b in range(B)]
    pmbb = [((mask[b].astype(f) - 1.0) * 1e4).astype(f) for b in range(B)]

    # cm[:, 0:128]: lower-triangle (p <= c) used for diagonal sub-tiles
    # tp=0..2; cm[:, 128:384]: tp=3 window over columns [256,512) of the
    # query block (zeros for c < 128+p, the all-masked strip, then triangle).
    cm = np.zeros((128, 384), f)
    pp, cc = np.meshgrid(np.arange(128), np.arange(128), indexing="ij")
    cm[:, 0:128] = (pp <= cc).astype(f)
    pp, cc = np.meshgrid(np.arange(128), np.arange(256), indexing="ij")
    cm[:, 128:384] = (cc >= 128 + pp).astype(f)

    ins = []
    for c in range(N_CORES):
        b, hg = c // 4, c % 4
        sl = slice(DS * hg, DS * (hg + 1))
        ins.append({
            "xT": xTb[b],
            "wq": np.ascontiguousarray(wqT[:, sl]),
            "wk": np.ascontiguousarray(wkT[:, sl]),
            "wv": np.ascontiguousarray(wvT[:, sl]),
            "wo": np.ascontiguousarray(woT[sl, :]),
            "bq": np.ascontiguousarray(bq8[sl]),
            "pmb": pmbb[b],
            "cmask": cm,
            "onesc": np.ones((1, HD), f),
        })
    return ins


def _run(ins, trace=False):
    from concourse.bass_utils import run_bass_kernel_spmd
    nc = _get_nc()
    return run_bass_kernel_spmd(nc, ins, list(range(N_CORES)), trace=trace)


def kernel(x, mask, Wq, bq, Wk, bk, Wv, bv, Wo, bo):
    ins = _make_inputs(x, mask, Wq, bq, Wk, bk, Wv, bv, Wo, bo)
    res = _run(ins)
    obias = (np.asarray(bo, np.float32)
             + np.asarray(Wo, np.float32) @ np.asarray(bv, np.float32))
    out = np.empty((B, S, D), np.float32)
    for b in range(B):
        acc = res.results[4 * b]["o"].astype(np.float32)
        for hg in range(1, 4):
            acc = acc + res.results[4 * b + hg]["o"]
        out[b] = acc.T + obias
    return out


# revision 27
# speedup vs baseline: 2.9768x; 1.2547x over previous
"""Multi-head self-attention (B=2, S=2048, D=1024, H=16, causal+padding mask)
on 8 Trainium2 NeuronCores via Bass/Tile, SPMD.

Sharding: core c -> batch b = c//4, head group hg = c%4 (heads 4hg..4hg+3,
i.e. a 256-wide slice of the model dim). Each core computes Q/K/V projections
only for its slice (no duplicated K/V work), blocked-causal attention for its
4 heads over all 2048 queries, and a row-parallel partial O-projection
Y_c = AT_c^T Wo_slice. The host sums the 4 partials per batch and adds the
output bias. Algebraic simplifications:
  - K bias is dropped: score(q,k) = Q_q.(xWk + bk)_k adds Q_q.bk, constant
    over k for fixed q, which softmax cancels.
  - V bias folds out: softmax rows sum to 1, so its contribution is
    bv @ Wo^T, a constant added on the host together with bo.
  - The 1/sqrt(64) score scale is folded into Wq/bq on the host.

Dataflow (per core, all transposed so no on-chip transposes are needed):
  x^T[d, s]     loaded once in 4 seq chunks of 512
  QT[dh, q]     = (Wq_sl x^T)*0.125 + bq*0.125   (chains of 8 matmuls, 512-free)
  KT[dh, k]     = Wk_sl x^T                       (no bias)
  V [k, dh+1]   = x Wv_sl^T with a ones column   (col 64 = softmax denominator)
  ST[k, q]      = KT_h^T QT_h per (head, 128-key tile, 512-query block)
  E             = exp(ST + padmask_bias); diagonal tiles *= causal01 (gpsimd)
  OT[dh+1, q]   += V_aug^T E                      (row 64 = denominators r)
  AT[dh, q]     = OT * (1/r)  (reciprocal of row 64, matmul-broadcast, DVE mul)
  Y^T[n, q]     = Wo_sl^T AT  partial, summed across cores on the host
Matmuls in float32r (full-rate fp32, ~1e-4 rel err). Softmax skips
max-subtraction: |scores| < ~5 so exp is safe; padding-masked keys get -1e4
added pre-exp which underflows to 0.

Schedule: attention for query block j is interleaved at (head-pair, key-tile)
granularity with the projection chains of seq chunk j+1 and the O-projection
of block j-1, keeping the PE stream continuous (the Act engine's exp
throughput is slightly below the PE's ST+PV rate, so pure attention would
stall the PE and drop it out of its high clock p-state).
"""

import sys

if "/opt/trn_rl_repo" not in sys.path:
    sys.path.insert(0, "/opt/trn_rl_repo")

import numpy as np

B, S, D, H, HD = 2, 2048, 1024, 16, 64
N_CORES = 8
HPC = 4              # heads per core
DS = HPC * HD        # 256: model-dim slice per core
QB = 512             # query block (free dim of ST/PV/proj matmuls)
NQB = S // QB        # 4
MC = D // 128        # 8 contraction chunks
NKT = S // 128       # 16 key tiles

_CACHE = {}


def _split_waits(nc, mybir):
    """This walrus build accepts only one sync-wait per instruction; move
    extra waits onto NOPs inserted just before, on the same engine."""
    n_new = 0
    for f in nc.m.functions:
        for blk in f.blocks:
            out = []
            for inst in blk.instructions:
                si = inst.sync_info
                if si is not None and si.on_wait is not None and len(si.on_wait) > 1:
                    waits = list(si.on_wait)
                    for w in waits[:-1]:
                        n_new += 1
                        out.append(mybir.InstNoOp(
                            name=f"I-waitsplit-{n_new}",
                            engine=inst.engine,
                            ins=[], outs=[],
                            sync_info=mybir.SyncInfo(on_wait=[w], on_update=[]),
                        ))
                    inst.sync_info = mybir.SyncInfo(
                        on_wait=[waits[-1]], on_update=list(si.on_update or []))
                out.append(inst)
            blk.instructions[:] = out
    return n_new


def _build():
    import concourse.bass as bass
    import concourse.mybir as mybir
    import concourse.tile as tile
    from contextlib import ExitStack

    f32 = mybir.dt.float32
    f32r = mybir.dt.float32r
    f16 = mybir.dt.float16
    EXP = mybir.ActivationFunctionType.Exp
    COPY = mybir.ActivationFunctionType.Copy
    MULT = mybir.AluOpType.mult
    ADD = mybir.AluOpType.add

    nc = bass.Bass()
    xT = nc.declare_dram_parameter("xT", [D, S], f16, isOutput=False)
    wq = nc.declare_dram_parameter("wq", [D, DS], f16, isOutput=False)
    wk = nc.declare_dram_parameter("wk", [D, DS], f16, isOutput=False)
    wv = nc.declare_dram_parameter("wv", [D, DS], f16, isOutput=False)
    wo = nc.declare_dram_parameter("wo", [DS, D], f16, isOutput=False)
    bq = nc.declare_dram_parameter("bq", [DS], f32, isOutput=False)
    pmb = nc.declare_dram_parameter("pmb", [S], f32, isOutput=False)
    cmask = nc.declare_dram_parameter("cmask", [128, 384], f16, isOutput=False)
    onesc = nc.declare_dram_parameter("onesc", [1, HD], f16, isOutput=False)
    out = nc.declare_dram_parameter("o", [D, S], f32, isOutput=True)

    with tile.TileContext(nc) as tc, ExitStack() as ctx, \
            nc.allow_low_precision("fp32r matmul inputs keep ~19 bits"):
        ec = ctx.enter_context
        consts = ec(tc.tile_pool(name="consts", bufs=1))
        big = ec(tc.tile_pool(name="big", bufs=1))
        e_p = ec(tc.tile_pool(name="e", bufs=6))
        rcp_p = ec(tc.tile_pool(name="rcp", bufs=2))
        yt_p = ec(tc.tile_pool(name="yt", bufs=2))
        proj_ps = ec(tc.tile_pool(name="proj_ps", bufs=2, space="PSUM"))
        st_ps = ec(tc.tile_pool(name="st_ps", bufs=4, space="PSUM"))
        ot_ps = ec(tc.tile_pool(name="ot_ps", bufs=2, space="PSUM"))

        # ---- constants into SBUF ----
        wq_sb = consts.tile([128, MC, DS], f16, tag="wq")
        nc.sync.dma_start(out=wq_sb, in_=wq.rearrange("(c p) n -> p c n", p=128))
        bq_sb = consts.tile([128, 2], f32, tag="bq")
        nc.sync.dma_start(out=bq_sb, in_=bq.rearrange("(c p) -> p c", p=128))
        pmb_sb = consts.tile([128, NKT], f32, tag="pmb")
        nc.sync.dma_start(out=pmb_sb, in_=pmb.rearrange("(t p) -> p t", p=128))

        xre = xT.rearrange("(c p) k -> p c k", p=128)
        x_sb = [big.tile([128, MC, QB], f16, tag=f"x{s}", name=f"x{s}")
                for s in range(NQB)]
        nc.sync.dma_start(out=x_sb[0], in_=xre[:, :, 0:QB])

        wk_sb = consts.tile([128, MC, DS], f16, tag="wk")
        nc.sync.dma_start(out=wk_sb, in_=wk.rearrange("(c p) n -> p c n", p=128))
        wv_sb = consts.tile([128, MC, DS], f16, tag="wv")
        nc.sync.dma_start(out=wv_sb, in_=wv.rearrange("(c p) n -> p c n", p=128))
        cm_sb = consts.tile([128, 384], f16, tag="cm")
        nc.sync.dma_start(out=cm_sb, in_=cmask[:, :])
        nc.sync.dma_start(out=x_sb[1], in_=xre[:, :, QB:2 * QB])
        wo_sb = consts.tile([128, 2, D], f16, tag="wo")
        nc.sync.dma_start(out=wo_sb, in_=wo.rearrange("(c p) n -> p c n", p=128))
        nc.sync.dma_start(out=x_sb[2], in_=xre[:, :, 2 * QB:3 * QB])
        nc.sync.dma_start(out=x_sb[3], in_=xre[:, :, 3 * QB:4 * QB])

        ones_sb = consts.tile([1, HD], f16, tag="ones")
        nc.sync.dma_start(out=ones_sb, in_=onesc[:, :])

        # persistent activations
        QT_sb = big.tile([128, 2, S], f16, tag="qt")         # 8KB/part
        KT_sb = big.tile([128, 2, S], f16, tag="kt")         # 8KB/part
        V_sb = big.tile([128, NKT, HPC, HD + 1], f16, tag="v")   # 8.3KB/part
        AT_sb = big.tile([128, 2, S], f16, tag="at")         # 8KB/part

        def proj_chunk_chains(s):
            """Return emit-closures, one per PSUM chain, for Q/K/V projection
            of seq chunk s (queries/keys [512s, 512s+512))."""
            chains = []
            for t in range(2):
                def qchain(t=t):
                    ps = proj_ps.tile([128, QB], f32, tag="ps")
                    for m in range(MC):
                        nc.tensor.matmul(
                            ps[:], wq_sb[:, m, t * 128:(t + 1) * 128],
                            x_sb[s][:, m, :], start=(m == 0), stop=(m == MC - 1))
                    nc.vector.tensor_scalar_add(
                        out=QT_sb[:, t, s * QB:(s + 1) * QB], in0=ps[:],
                        scalar1=bq_sb[:, t:t + 1])
                chains.append(qchain)
            for t in range(2):
                def kchain(t=t):
                    ps = proj_ps.tile([128, QB], f32, tag="ps")
                    for m in range(MC):
                        nc.tensor.matmul(
                            ps[:], wk_sb[:, m, t * 128:(t + 1) * 128],
                            x_sb[s][:, m, :], start=(m == 0), stop=(m == MC - 1))
                    nc.vector.tensor_copy(
                        KT_sb[:, t, s * QB:(s + 1) * QB], ps[:])
                chains.append(kchain)
            for ktl in range(4):
                def vchain(ktl=ktl):
                    kt = 4 * s + ktl
                    ps = proj_ps.tile([128, QB], f32, tag="ps")
                    for m in range(MC):
                        nc.tensor.matmul(
                            ps[:, 0:DS], x_sb[s][:, m, ktl * 128:(ktl + 1) * 128],
                            wv_sb[:, m, :], start=(m == 0), stop=(m == MC - 1))
                    nc.vector.tensor_copy(
                        V_sb[:, kt, :, 0:HD],
                        ps[:, 0:DS].rearrange("p (h d) -> p h d", d=HD))
                    nc.vector.tensor_scalar(
                        out=V_sb[:, kt, :, HD:HD + 1],
                        in0=ps[:, 0:DS].rearrange(
                            "p (h d) -> p h d", d=HD)[:, :, 0:1],
                        scalar1=0.0, scalar2=1.0, op0=MULT, op1=ADD)
                chains.append(vchain)
            return chains

        def oproj_chains(j):
            """Partial output projection for query block j (AT must be done)."""
            chains = []
            for nt in range(MC):
                def ochain(nt=nt):
                    ps = proj_ps.tile([128, QB], f32, tag="ps")
                    for c in range(2):
                        nc.tensor.matmul(
                            ps[:], wo_sb[:, c, nt * 128:(nt + 1) * 128],
                            AT_sb[:, c, j * QB:(j + 1) * QB],
                            start=(c == 0), stop=(c == 1))
                    yt = yt_p.tile([128, QB], f32, tag="yt")
                    nc.vector.tensor_copy(yt[:], ps[:])
                    nc.sync.dma_start(
                        out=out[nt * 128:(nt + 1) * 128, j * QB:(j + 1) * QB],
                        in_=yt[:])
                chains.append(ochain)
            return chains

        def attention_block(j, fillers):
            """Attention for query block j (all 4 heads, processed as 2 pairs).

            Software-pipelined one key-tile ahead: the PV pair for tile kt is
            emitted after the ST/exp pair for tile kt+1, so exp latency hides
            behind other PE work. Filler emit-closures (projection/O-proj
            chains) are spread between steps to cover the Act engine's lower
            throughput. Diagonal tiles only compute/exp/mask the causally
            reachable column range [c0, 512): columns below c0 see none of the
            tile's keys, and the mask multiply only covers the partial window.
            """
            nkt = 4 * (j + 1)
            nfill = len(fillers)
            fi = 0
            nsteps = 2 * (nkt + 1)
            si = 0
            es = {}

            def c0_of(kt):
                tp = kt - 4 * j
                return 0 if tp < 0 else min(128 * tp, 256)

            def st_exp(p, kt):
                c0 = c0_of(kt)
                tp = kt - 4 * j
                for hi in range(2):
                    h = 2 * p + hi
                    pr, hw = h // 2, 64 * (h % 2)
                    st = st_ps.tile([128, QB], f32, tag="st")
                    nc.tensor.matmul(
                        st[:, c0:],
                        KT_sb[hw:hw + 64, pr, kt * 128:(kt + 1) * 128],
                        QT_sb[hw:hw + 64, pr, j * QB + c0:(j + 1) * QB],
                        start=True, stop=True)
                    e = e_p.tile([128, QB], f16, tag="e")
                    nc.scalar.activation(out=e[:, c0:], in_=st[:, c0:],
                                         func=EXP, bias=pmb_sb[:, kt:kt + 1])
                    if tp >= 0:
                        if tp < 3:
                            nc.gpsimd.tensor_mul(
                                e[:, 128 * tp:128 * (tp + 1)],
                                e[:, 128 * tp:128 * (tp + 1)],
                                cm_sb[:, 0:128])
                        else:
                            nc.gpsimd.tensor_mul(
                                e[:, 256:512], e[:, 256:512],
                                cm_sb[:, 128:384])
                    es[(p, kt, hi)] = e

            def pv(p, kt):
                c0 = c0_of(kt)
                for hi in range(2):
                    h = 2 * p + hi
                    nc.tensor.matmul(
                        ots[p][hi][:, c0:], V_sb[:, kt, h, :],
                        es.pop((p, kt, hi))[:, c0:],
                        start=(kt == 0), stop=(kt == nkt - 1))

            ots = {}
            for p in range(2):
                ots[p] = [ot_ps.tile([HD + 1, QB], f32, tag="ot",
                                     name=f"ot{j}_{p}_{hi2}")
                          for hi2 in range(2)]
                for kt in range(nkt):
                    st_exp(p, kt)
                    if kt >= 1:
                        pv(p, kt - 1)
                    si += 1
                    want = si * nfill // nsteps
                    while fi < want:
                        fillers[fi]()
                        fi += 1
                pv(p, nkt - 1)
                si += 1
                for hi in range(2):
                    h = 2 * p + hi
                    pr, hw = h // 2, 64 * (h % 2)
                    otp = ots[p][hi]
                    dn = rcp_p.tile([1, QB], f32, tag="dn")
                    nc.vector.tensor_copy(dn[:], otp[HD:HD + 1, :])
                    rcp = rcp_p.tile([1, QB], f32, tag="rcp")
                    nc.vector.reciprocal_approx_fast(out=rcp[:], in_=dn[:])
                    rcpr = rcp_p.tile([1, QB], f16, tag="rcpr")
                    nc.vector.tensor_copy(rcpr[:], rcp[:])
                    bc = st_ps.tile([128, QB], f32, tag="st")
                    nc.tensor.matmul(bc[0:HD, :], ones_sb[:], rcpr[:],
                                     start=True, stop=True)
                    rb = rcp_p.tile([HD, QB], f16, tag="rb")
                    nc.scalar.activation(out=rb[:], in_=bc[0:HD, :],
                                         func=COPY)
                    nc.vector.tensor_mul(
                        AT_sb[hw:hw + 64, pr, j * QB:(j + 1) * QB],
                        otp[0:HD, :], rb[:])
            while fi < nfill:
                fillers[fi]()
                fi += 1

        # ---- schedule ----
        for ch in proj_chunk_chains(0):
            ch()
        attention_block(0, proj_chunk_chains(1))
        attention_block(1, proj_chunk_chains(2))
        attention_block(2, proj_chunk_chains(3))
        attention_block(3, oproj_chains(0) + oproj_chains(1) + oproj_chains(2))
        for ch in oproj_chains(3):
            ch()

    from concourse.library_overlay import lower_extended_insts
    lower_extended_insts(nc)
    _split_waits(nc, mybir)
    return nc


def _get_nc():
    if "nc" not in _CACHE:
        _CACHE["nc"] = _build()
    return _CACHE["nc"]


def _make_inputs(x, mask, Wq, bq, Wk, bk, Wv, bv, Wo, bo):
    f = np.float32
    x = np.asarray(x, f)
    mask = np.asarray(mask)
    Wq, bq = np.asarray(Wq, f), np.asarray(bq, f)
    Wk = np.asarray(Wk, f)
    Wv = np.asarray(Wv, f)
    Wo = np.asarray(Wo, f)

    h = np.float16
    wqT = (np.ascontiguousarray(Wq.T) * np.float32(0.125)).astype(h)
    wkT = np.ascontiguousarray(Wk.T).astype(h)
    wvT = np.ascontiguousarray(Wv.T).astype(h)
    woT = np.ascontiguousarray(Wo.T).astype(h)
    bq8 = (bq * 0.125).astype(f)

    xTb = [np.ascontiguousarray(x[b].T).astype(h) for b in range(B)]
    pmbb = [((mask[b].astype(f) - 1.0) * 1e4).astype(f) for b in range(B)]

    # cm[:, 0:128]: lower-triangle (p <= c) used for diagonal sub-tiles
    # tp=0..2; cm[:, 128:384]: tp=3 window over columns [256,512) of the
    # query block (zeros for c < 128+p, the all-masked strip, then triangle).
    cm = np.zeros((128, 384), h)
    pp, cc = np.meshgrid(np.arange(128), np.arange(128), indexing="ij")
    cm[:, 0:128] = (pp <= cc).astype(h)
    pp, cc = np.meshgrid(np.arange(128), np.arange(256), indexing="ij")
    cm[:, 128:384] = (cc >= 128 + pp).astype(h)

    ins = []
    for c in range(N_CORES):
        b, hg = c // 4, c % 4
        sl = slice(DS * hg, DS * (hg + 1))
        ins.append({
            "xT": xTb[b],
            "wq": np.ascontiguousarray(wqT[:, sl]),
            "wk": np.ascontiguousarray(wkT[:, sl]),
            "wv": np.ascontiguousarray(wvT[:, sl]),
            "wo": np.ascontiguousarray(woT[sl, :]),
            "bq": np.ascontiguousarray(bq8[sl]),
            "pmb": pmbb[b],
            "cmask": cm,
            "onesc": np.ones((1, HD), np.float16),
        })
    return ins


def _run(ins, trace=False):
    from concourse.bass_utils import run_bass_kernel_spmd
    nc = _get_nc()
    return run_bass_kernel_spmd(nc, ins, list(range(N_CORES)), trace=trace)


def kernel(x, mask, Wq, bq, Wk, bk, Wv, bv, Wo, bo):
    ins = _make_inputs(x, mask, Wq, bq, Wk, bk, Wv, bv, Wo, bo)
    res = _run(ins)
    obias = (np.asarray(bo, np.float32)
             + np.asarray(Wo, np.float32) @ np.asarray(bv, np.float32))
    out = np.empty((B, S, D), np.float32)
    for b in range(B):
        acc = res.results[4 * b]["o"].astype(np.float32)
        for hg in range(1, 4):
            acc = acc + res.results[4 * b + hg]["o"]
        out[b] = acc.T + obias
    return out
